# revision 30
# baseline (speedup 1.0000x reference)
"""Trainium2 Bass kernel for nn_AttentionModel (B=262144, C=256, P=100).

  alpha[b] = sum_p w[p] * tanh(u[p]@f[b]) * sigmoid(v[p]@f[b]);  out = softmax(alpha)

Strategy (8 cores, data-parallel over B, ~48-52 us/core steady state):
  - Host casts features to fp16 and PRE-TRANSPOSES the two 128-column halves
    to [128c, BS b] per core: device DMAs are plain contiguous HBM reads
    (~420 GB/s) instead of DMA-transpose (~255 GB/s).
  - Chunk schedule is tapered (1K,1K,2K,4K*6,2K,1K,1K rows) to shrink the
    pipeline ramp (first-chunk DMA) and drain (last-chunk compute tail).
  - PE: per 128-b tile, two LDW+MM fp16 pairs (stationary = f.T chunk,
    moving = [u.T | v.T] 128x200) accumulating PSUM [128b, 200]. ~46 us.
  - ACT: tanh / sigmoid PSUM->SBUF fp16, one op per 8-tile PSUM group per
    function (FD=800; the 172-cycle per-op bubble makes this the binding
    engine at ~52 us busy).
  - DVE: prod=tu*tv, prodw=prod*w (w pre-tiled dense [128,fuse*G*P] so
    tensor_tensor runs 2x), then a pairwise fold 100->50->25 at 2x before
    the 1x-only tensor_reduce (DVE ~48 us vs 56 unfolded).
  - Softmax: max-free (|alpha| < 85 so exp stays in fp32 range): one
    exp+accum pass, PE column-sum matmul, ONE 8-core AllReduce(add) of the
    scalar sum (~12 us ncfw launch overhead, the dominant serial tail),
    PE broadcast matmul, one scale pass, output DMA.
"""

import numpy as np

import concourse.bass as bass
import concourse.mybir as mybir
import concourse.tile as tile
from concourse import bacc, bass_isa
from concourse.bass_utils import run_bass_kernel_spmd

B = 262144
C = 256
P = 100
NCORES = 8
BS = B // NCORES          # 32768 rows per core
NT = 4096                 # rows per transpose-DMA chunk
NCHUNK = BS // NT         # 8
G = 8                     # 128-row tiles per PSUM group
GPC = NT // (128 * G)     # groups per chunk = 4
TPC = BS // 128           # tiles per core = 256
FUSE = 2                  # groups batched per DVE op

F16 = mybir.dt.float16
F32 = mybir.dt.float32
AF = mybir.ActivationFunctionType
ALU = mybir.AluOpType


def _build(n_cores: int = NCORES, use_collective: bool = True,
           parts: str = "dma,mm,act,dve,p2", repeats: int = 1,
           g: int = G, nt: int = NT, ftbufs: int = 5, actbufs: int = 7,
           psbufs: int = 2, fuse: int = 2, layout: str = "flat",
           dvemode: str = "fold", taper: int = 1, p2v: int = 2) -> bass.Bass:
    nc = bacc.Bacc(
        "TRN2",
        target_bir_lowering=False,
        debug=False,
        num_devices=n_cores,
    )
    if layout == "flat":
        f0 = nc.dram_tensor("f0", [128, BS], F16, kind="ExternalInput").ap()
        f1 = nc.dram_tensor("f1", [128, BS], F16, kind="ExternalInput").ap()
    else:
        f0 = nc.dram_tensor("f0", [BS, 128], F16, kind="ExternalInput").ap()
        f1 = nc.dram_tensor("f1", [BS, 128], F16, kind="ExternalInput").ap()
    uv = nc.dram_tensor("uv", [2, 128, 2 * P], F16, kind="ExternalInput").ap()
    wb = nc.dram_tensor("wb", [128, fuse * g * P], F16, kind="ExternalInput").ap()
    out_t = nc.dram_tensor("out", [128, TPC], F32, kind="ExternalOutput").ap()

    with tile.TileContext(nc) as tc:
        _body(nc, tc, f0, f1, uv, wb, out_t, n_cores, use_collective,
              frozenset(parts.split(",")), repeats, g, nt, ftbufs, actbufs,
              psbufs, fuse, layout, dvemode, taper, p2v)
    nc.compile()
    return nc


def _body(nc, tc, f0, f1, uv, wb, out_t, n_cores, use_collective=True,
          parts=frozenset({"dma", "mm", "act", "dve", "p2"}), repeats=1,
          g=G, nt=NT, ftbufs=3, actbufs=3, psbufs=2, fuse=1, layout="tr",
          dvemode="3op", taper=1, p2v=2):
    G_, NT_ = g, nt
    NCHUNK_ = BS // NT_
    GPC_ = NT_ // (128 * G_)
    assert GPC_ % fuse == 0
    with (
        tc.tile_pool(name="const", bufs=1) as constp,
        tc.tile_pool(name="alpha", bufs=1) as alphap,
    ):
        if "mm" in parts:
            uv_sb = constp.tile([128, 2, 2 * P], F16)
            nc.scalar.dma_start(uv_sb, uv.rearrange("k p m -> p k m"))
        if "dve" in parts:
            wb_sb = constp.tile([128, fuse, G_, P], F16)
            nc.scalar.dma_start(wb_sb, wb.rearrange("p (f g m) -> p f g m",
                                                    f=fuse, g=G_))
        alpha_sb = alphap.tile([128, TPC], F32)
        if "dve" not in parts:
            nc.vector.memset(alpha_sb, 0.0)

        # ---------------- phase 1: alpha ----------------
        with (
            tc.tile_pool(name="ft", bufs=ftbufs) as ftp,
            tc.tile_pool(name="acts", bufs=actbufs) as actp,
            tc.tile_pool(name="ps", bufs=psbufs, space="PSUM") as psp,
        ):
            if taper and NT_ == 4096:
                sched = [1024, 1024, 2048] + [4096] * 6 + [2048, 1024, 1024]
            else:
                sched = [NT_] * NCHUNK_
            assert sum(sched) == BS
            offs = [sum(sched[:i]) for i in range(len(sched))]
            gidx = 0
            for off, sz in list(zip(offs, sched)) * repeats:
                ft0 = ftp.tile([128, NT_], F16, tag="ft0")
                ft1 = ftp.tile([128, NT_], F16, tag="ft1")
                if "dma" in parts:
                    if layout == "flat":
                        nc.sync.dma_start(ft0[:, 0:sz], f0[:, off:off + sz])
                        nc.sync.dma_start(ft1[:, 0:sz], f1[:, off:off + sz])
                    else:
                        nc.sync.dma_start_transpose(ft0[:, 0:sz],
                                                    f0[off:off + sz, :])
                        nc.sync.dma_start_transpose(ft1[:, 0:sz],
                                                    f1[off:off + sz, :])
                elif "mm" in parts:
                    # timing-ablation only: tiny write so the tiles allocate
                    nc.vector.memset(ft0[:, 0:1], 0.0)
                    nc.vector.memset(ft1[:, 0:1], 0.0)
                for gi in range(sz // (128 * G_)):
                    if "mm" not in parts:
                        continue
                    ps = psp.tile([128, G_, 256], F32, tag="ps")
                    for j in range(G_):
                        col = (gi * G_ + j) * 128
                        assert col + 128 <= NT_
                        nc.tensor.matmul(
                            ps[:, j, 0:2 * P],
                            lhsT=ft0[:, col:col + 128],
                            rhs=uv_sb[:, 0],
                            start=True, stop=False,
                        )
                        nc.tensor.matmul(
                            ps[:, j, 0:2 * P],
                            lhsT=ft1[:, col:col + 128],
                            rhs=uv_sb[:, 1],
                            start=False, stop=True,
                        )
                    if "act" not in parts:
                        gidx += 1
                        continue
                    gpp = BS // (128 * G_)       # groups per pass
                    bi = gidx % fuse
                    if bi == 0:
                        tub = actp.tile([128, fuse, G_, P], F16, tag="tu")
                        tvb = actp.tile([128, fuse, G_, P], F16, tag="tv")
                    nc.scalar.activation(tub[:, bi], ps[:, :, 0:P], AF.Tanh)
                    nc.scalar.activation(tvb[:, bi], ps[:, :, P:2 * P],
                                         AF.Sigmoid)
                    gidx += 1
                    if "dve" in parts and bi == fuse - 1:
                        ti = ((gidx - fuse) % gpp) * G_
                        if dvemode == "ttr":
                            # tvw = tv*w, then fused (tu*tvw -> sum_p) per tile
                            tvw = actp.tile([128, fuse, G_, P], F16, tag="tvw")
                            nc.vector.tensor_tensor(tvw, tvb, wb_sb, ALU.mult)
                            scr = actp.tile([128, fuse, G_, P], F16, tag="scr")
                            for f in range(fuse):
                                for j in range(G_):
                                    nc.vector.tensor_tensor_reduce(
                                        scr[:, f, j], tub[:, f, j], tvw[:, f, j],
                                        1.0, 0.0, ALU.mult, ALU.add,
                                        alpha_sb[:, ti + f * G_ + j:
                                                 ti + f * G_ + j + 1],
                                    )
                        elif dvemode == "fold":
                            prod = actp.tile([128, fuse, G_, P], F16, tag="prod")
                            nc.vector.tensor_tensor(prod, tub, tvb, ALU.mult)
                            prodw = actp.tile([128, fuse, G_, P], F16, tag="prodw")
                            nc.vector.tensor_tensor(prodw, prod, wb_sb, ALU.mult)
                            h1 = actp.tile([128, fuse, G_, 50], F16, tag="h1")
                            nc.vector.tensor_tensor(
                                h1, prodw[:, :, :, 0:50], prodw[:, :, :, 50:100],
                                ALU.add)
                            h2 = actp.tile([128, fuse, G_, 25], F16, tag="h2")
                            nc.vector.tensor_tensor(
                                h2, h1[:, :, :, 0:25], h1[:, :, :, 25:50],
                                ALU.add)
                            nc.vector.tensor_reduce(
                                alpha_sb[:, ti:ti + fuse * G_], h2,
                                axis=mybir.AxisListType.X, op=ALU.add,
                            )
                        else:
                            prod = actp.tile([128, fuse, G_, P], F16, tag="prod")
                            nc.vector.tensor_tensor(prod, tub, tvb, ALU.mult)
                            prodw = actp.tile([128, fuse, G_, P], F16, tag="prodw")
                            nc.vector.tensor_tensor(prodw, prod, wb_sb, ALU.mult)
                            nc.vector.tensor_reduce(
                                alpha_sb[:, ti:ti + fuse * G_], prodw,
                                axis=mybir.AxisListType.X, op=ALU.add,
                            )

        # ---------------- phase 2: softmax ----------------
        if "p2" not in parts:
            with tc.tile_pool(name="p2x", bufs=1) as p2x:
                dummy = p2x.tile([128, TPC], F32)
                nc.vector.tensor_copy(dummy, alpha_sb)
                nc.sync.dma_start(out_t, dummy)
            return
        with (
            tc.tile_pool(name="p2", bufs=1) as p2,
            tc.tile_pool(name="p2ps", bufs=1, space="PSUM") as p2ps,
            tc.tile_pool(name="dram", bufs=1, space="DRAM") as dramp,
        ):
            def phase2_v2():
                # softmax without max-subtraction: |alpha| <~ sum|w| ~ 80, so
                # exp(alpha) and the per-core sums stay well inside fp32 range
                # (validated against the reference inputs in test.py).
                e_sb = p2.tile([128, TPC], F32, tag="e_sb")
                sums = p2.tile([128, 1], F32, tag="sums")
                nc.scalar.activation(e_sb, alpha_sb, AF.Exp, accum_out=sums)
                onesP = p2.tile([128, 1], F32, tag="onesP")
                nc.vector.memset(onesP, 1.0)
                s_ps = p2ps.tile([1, 1], F32, tag="s_ps")
                nc.tensor.matmul(s_ps, lhsT=sums, rhs=onesP,
                                 start=True, stop=True)
                s_sb = p2.tile([1, 1], F32, tag="s_sb")
                nc.vector.tensor_copy(s_sb, s_ps)
                s_tot = p2.tile([1, 1], F32, tag="s_tot")
                if use_collective:
                    cin = dramp.tile([1, 1], F32, tag="cin")
                    cout = dramp.tile([1, 1], F32, tag="cout")
                    nc.sync.dma_start(cin, s_sb)
                    nc.gpsimd.collective_compute(
                        "AllReduce",
                        ALU.add,
                        ins=[cin.opt()],
                        outs=[cout.opt()],
                        replica_groups=[list(range(n_cores))],
                    )
                    nc.sync.dma_start(s_tot, cout)
                else:
                    nc.vector.tensor_scalar_mul(s_tot, s_sb, float(n_cores))
                r_s = p2.tile([1, 1], F32, tag="r_s")
                nc.vector.reciprocal(r_s, s_tot)
                ones1 = p2.tile([1, 128], F32, tag="ones1")
                nc.vector.memset(ones1, 1.0)
                c_ps = p2ps.tile([128, 1], F32, tag="c_ps")
                nc.tensor.matmul(c_ps, lhsT=ones1, rhs=r_s,
                                 start=True, stop=True)
                c128 = p2.tile([128, 1], F32, tag="c128")
                nc.vector.tensor_copy(c128, c_ps)
                outt = p2.tile([128, TPC], F32, tag="outt")
                nc.vector.tensor_scalar_mul(outt, e_sb, c128)
                nc.sync.dma_start(out_t, outt)

            def phase2():
                mx = p2.tile([128, 1], F32, tag="mx")
                nc.vector.reduce_max(mx, alpha_sb, axis=mybir.AxisListType.X)
                mxr = p2.tile([128, 1], F32, tag="mxr")
                nc.gpsimd.partition_all_reduce(
                    mxr, mx, channels=128, reduce_op=bass_isa.ReduceOp.max
                )
                negm = p2.tile([128, 1], F32, tag="negm")
                nc.vector.tensor_scalar_mul(negm, mxr, -1.0)
                e_sb = p2.tile([128, TPC], F32, tag="e_sb")
                sums = p2.tile([128, 1], F32, tag="sums")
                nc.scalar.activation(e_sb, alpha_sb, AF.Exp, bias=negm,
                                     accum_out=sums)
                sr = p2.tile([128, 1], F32, tag="sr")
                nc.gpsimd.partition_all_reduce(
                    sr, sums, channels=128, reduce_op=bass_isa.ReduceOp.add
                )
                # pack (m_local, s_local) and all-gather across cores
                ms = p2.tile([1, 2], F32, tag="ms")
                nc.vector.tensor_copy(ms[:, 0:1], mxr[0:1, :])
                nc.vector.tensor_copy(ms[:, 1:2], sr[0:1, :])
                gath = p2.tile([1, 2, n_cores], F32, tag="gath")
                if use_collective:
                    cin = dramp.tile([1, 2], F32, tag="cin")
                    cout = dramp.tile([n_cores, 2], F32, tag="cout")
                    nc.sync.dma_start(cin, ms)
                    nc.gpsimd.collective_compute(
                        "AllGather",
                        ALU.bypass,
                        ins=[cin.opt()],
                        outs=[cout.opt()],
                        replica_groups=[list(range(n_cores))],
                    )
                    nc.sync.dma_start(gath, cout.rearrange("i two -> two i"))
                else:
                    # single-core debug: replicate local (m, s) n_cores times
                    for i in range(n_cores):
                        nc.vector.tensor_copy(gath[:, :, i], ms)
                mg = p2.tile([1, 1], F32, tag="mg")
                nc.vector.reduce_max(mg, gath[:, 0], axis=mybir.AxisListType.X)
                neg_mg = p2.tile([1, 1], F32, tag="neg_mg")
                nc.vector.tensor_scalar_mul(neg_mg, mg, -1.0)
                e8 = p2.tile([1, n_cores], F32, tag="e8")
                nc.scalar.activation(e8, gath[:, 0], AF.Exp, bias=neg_mg)
                p8 = p2.tile([1, n_cores], F32, tag="p8")
                nc.vector.tensor_tensor(p8, e8, gath[:, 1], ALU.mult)
                s_tot = p2.tile([1, 1], F32, tag="s_tot")
                nc.vector.reduce_sum(s_tot, p8, axis=mybir.AxisListType.X)
                r_s = p2.tile([1, 1], F32, tag="r_s")
                nc.vector.reciprocal(r_s, s_tot)
                eml = p2.tile([1, 1], F32, tag="eml")
                nc.scalar.activation(eml, mxr[0:1, :], AF.Exp, bias=neg_mg)
                c1 = p2.tile([1, 1], F32, tag="c1")
                nc.vector.tensor_tensor(c1, eml, r_s, ALU.mult)
                c128 = p2.tile([128, 1], F32, tag="c128")
                nc.gpsimd.partition_broadcast(c128, c1)
                outt = p2.tile([128, TPC], F32, tag="outt")
                nc.vector.tensor_scalar_mul(outt, e_sb, c128)
                nc.sync.dma_start(out_t, outt)

            for _ in range(repeats if "p2rep" in parts else 1):
                phase2_v2() if p2v == 2 else phase2()


_CACHE: dict = {}


def _get_nc() -> bass.Bass:
    if "nc" not in _CACHE:
        _CACHE["nc"] = _build(NCORES)
    return _CACHE["nc"]


def kernel(features: np.ndarray, u: np.ndarray, v: np.ndarray, w: np.ndarray,
           **_unused) -> np.ndarray:
    features = np.asarray(features)
    u, v, w = np.asarray(u), np.asarray(v), np.asarray(w)
    assert features.shape == (B, C)
    f16 = features.astype(np.float16)
    uvt = np.ascontiguousarray(
        np.concatenate([u.T, v.T], axis=1).astype(np.float16)
    )  # [C, 2P]
    uv_arr = np.ascontiguousarray(uvt.reshape(2, 128, 2 * P))
    wb = np.ascontiguousarray(
        np.tile(w.reshape(-1).astype(np.float16), (128, FUSE * G))
    )  # [128, FUSE*G*P]
    in_maps = []
    for i in range(NCORES):
        sl = slice(i * BS, (i + 1) * BS)
        in_maps.append({
            "f0": np.ascontiguousarray(f16[sl, :128].T),
            "f1": np.ascontiguousarray(f16[sl, 128:].T),
            "uv": uv_arr,
            "wb": wb,
        })
    res = run_bass_kernel_spmd(_get_nc(), in_maps, core_ids=list(range(NCORES)))
    outs = [r["out"] for r in res.results]
    return np.concatenate([o.T.reshape(-1) for o in outs]).astype(np.float32)



# revision 32
# speedup vs baseline: 1.0845x; 1.0845x over previous
"""Trainium2 Bass kernel for nn_AttentionModel (B=262144, C=256, P=100).

  alpha[b] = sum_p w[p] * tanh(u[p]@f[b]) * sigmoid(v[p]@f[b]);  out = softmax(alpha)

Strategy (8 cores, data-parallel over B, ~48-52 us/core steady state):
  - Host casts features to fp16 and PRE-TRANSPOSES the two 128-column halves
    to [128c, BS b] per core: device DMAs are plain contiguous HBM reads
    (~420 GB/s) instead of DMA-transpose (~255 GB/s).
  - Chunk schedule is tapered (1K,1K,2K,4K*6,2K,1K,1K rows) to shrink the
    pipeline ramp (first-chunk DMA) and drain (last-chunk compute tail).
  - PE: per 128-b tile, two LDW+MM fp16 pairs (stationary = f.T chunk,
    moving = [u.T | v.T] 128x200) accumulating PSUM [128b, 200]. ~46 us.
  - ACT: tanh / sigmoid PSUM->SBUF fp16, one op per 8-tile PSUM group per
    function (FD=800; the 172-cycle per-op bubble makes this the binding
    engine at ~52 us busy).
  - DVE: prod=tu*tv, prodw=prod*w (w pre-tiled dense [128,fuse*G*P] so
    tensor_tensor runs 2x), then a pairwise fold 100->50->25 at 2x before
    the 1x-only tensor_reduce (DVE ~48 us vs 56 unfolded).
  - Softmax: max-free (|alpha| < 85 so exp stays in fp32 range): one
    exp+accum pass, PE column-sum matmul, ONE 8-core AllReduce(add) of the
    scalar sum (~12 us ncfw launch overhead, the dominant serial tail),
    PE broadcast matmul, one scale pass, output DMA.
"""

import numpy as np

import concourse.bass as bass
import concourse.mybir as mybir
import concourse.tile as tile
from concourse import bacc, bass_isa
from concourse.bass_utils import run_bass_kernel_spmd

B = 262144
C = 256
P = 100
NCORES = 8
BS = B // NCORES          # 32768 rows per core
NT = 4096                 # rows per transpose-DMA chunk
NCHUNK = BS // NT         # 8
G = 8                     # 128-row tiles per PSUM group
GPC = NT // (128 * G)     # groups per chunk = 4
TPC = BS // 128           # tiles per core = 256
FUSE = 4                  # groups batched per DVE op

F16 = mybir.dt.float16
F32 = mybir.dt.float32
AF = mybir.ActivationFunctionType
ALU = mybir.AluOpType


def _build(n_cores: int = NCORES, use_collective: bool = True,
           parts: str = "dma,mm,act,dve,p2", repeats: int = 1,
           g: int = G, nt: int = NT, ftbufs: int = 5, actbufs: int = 4,
           psbufs: int = 2, fuse: int = FUSE, layout: str = "flat",
           dvemode: str = "fold", taper: int = 1, p2v: int = 2) -> bass.Bass:
    nc = bacc.Bacc(
        "TRN2",
        target_bir_lowering=False,
        debug=False,
        num_devices=n_cores,
    )
    if layout == "flat":
        f0 = nc.dram_tensor("f0", [128, BS], F16, kind="ExternalInput").ap()
        f1 = nc.dram_tensor("f1", [128, BS], F16, kind="ExternalInput").ap()
    else:
        f0 = nc.dram_tensor("f0", [BS, 128], F16, kind="ExternalInput").ap()
        f1 = nc.dram_tensor("f1", [BS, 128], F16, kind="ExternalInput").ap()
    uv = nc.dram_tensor("uv", [2, 128, 2 * P], F16, kind="ExternalInput").ap()
    wb = nc.dram_tensor("wb", [128, fuse * g * P], F16, kind="ExternalInput").ap()
    out_t = nc.dram_tensor("out", [128, TPC], F32, kind="ExternalOutput").ap()

    with tile.TileContext(nc) as tc:
        _body(nc, tc, f0, f1, uv, wb, out_t, n_cores, use_collective,
              frozenset(parts.split(",")), repeats, g, nt, ftbufs, actbufs,
              psbufs, fuse, layout, dvemode, taper, p2v)
    nc.compile()
    return nc


def _body(nc, tc, f0, f1, uv, wb, out_t, n_cores, use_collective=True,
          parts=frozenset({"dma", "mm", "act", "dve", "p2"}), repeats=1,
          g=G, nt=NT, ftbufs=3, actbufs=3, psbufs=2, fuse=1, layout="tr",
          dvemode="3op", taper=1, p2v=2):
    G_, NT_ = g, nt
    NCHUNK_ = BS // NT_
    GPC_ = NT_ // (128 * G_)
    assert GPC_ % fuse == 0
    with (
        tc.tile_pool(name="const", bufs=1) as constp,
        tc.tile_pool(name="alpha", bufs=1) as alphap,
    ):
        if "mm" in parts:
            uv_sb = constp.tile([128, 2, 2 * P], F16)
            nc.scalar.dma_start(uv_sb, uv.rearrange("k p m -> p k m"))
        if "dve" in parts:
            wb_sb = constp.tile([128, fuse, G_, P], F16)
            nc.scalar.dma_start(wb_sb, wb.rearrange("p (f g m) -> p f g m",
                                                    f=fuse, g=G_))
        alpha_sb = alphap.tile([128, TPC], F32)
        if "dve" not in parts:
            nc.vector.memset(alpha_sb, 0.0)

        # ---------------- phase 1: alpha ----------------
        with (
            tc.tile_pool(name="ft", bufs=ftbufs) as ftp,
            tc.tile_pool(name="acts", bufs=actbufs) as actp,
            tc.tile_pool(name="ps", bufs=psbufs, space="PSUM") as psp,
        ):
            if taper and NT_ == 4096:
                sched = [1024, 1024, 2048] + [4096] * 6 + [2048, 1024, 1024]
            else:
                sched = [NT_] * NCHUNK_
            assert sum(sched) == BS
            offs = [sum(sched[:i]) for i in range(len(sched))]
            gidx = 0
            for off, sz in list(zip(offs, sched)) * repeats:
                ft0 = ftp.tile([128, NT_], F16, tag="ft0")
                ft1 = ftp.tile([128, NT_], F16, tag="ft1")
                if "dma" in parts:
                    if layout == "flat":
                        nc.sync.dma_start(ft0[:, 0:sz], f0[:, off:off + sz])
                        nc.sync.dma_start(ft1[:, 0:sz], f1[:, off:off + sz])
                    else:
                        nc.sync.dma_start_transpose(ft0[:, 0:sz],
                                                    f0[off:off + sz, :])
                        nc.sync.dma_start_transpose(ft1[:, 0:sz],
                                                    f1[off:off + sz, :])
                elif "mm" in parts:
                    # timing-ablation only: tiny write so the tiles allocate
                    nc.vector.memset(ft0[:, 0:1], 0.0)
                    nc.vector.memset(ft1[:, 0:1], 0.0)
                for gi in range(sz // (128 * G_)):
                    if "mm" not in parts:
                        continue
                    ps = psp.tile([128, G_, 256], F32, tag="ps")
                    for j in range(G_):
                        col = (gi * G_ + j) * 128
                        assert col + 128 <= NT_
                        nc.tensor.matmul(
                            ps[:, j, 0:2 * P],
                            lhsT=ft0[:, col:col + 128],
                            rhs=uv_sb[:, 0],
                            start=True, stop=False,
                        )
                        nc.tensor.matmul(
                            ps[:, j, 0:2 * P],
                            lhsT=ft1[:, col:col + 128],
                            rhs=uv_sb[:, 1],
                            start=False, stop=True,
                        )
                    if "act" not in parts:
                        gidx += 1
                        continue
                    gpp = BS // (128 * G_)       # groups per pass
                    bi = gidx % fuse
                    if bi == 0:
                        tub = actp.tile([128, fuse, G_, P], F16, tag="tu")
                        tvb = actp.tile([128, fuse, G_, P], F16, tag="tv")
                    nc.scalar.activation(tub[:, bi], ps[:, :, 0:P], AF.Tanh)
                    nc.scalar.activation(tvb[:, bi], ps[:, :, P:2 * P],
                                         AF.Sigmoid)
                    gidx += 1
                    if "dve" in parts and bi == fuse - 1:
                        ti = ((gidx - fuse) % gpp) * G_
                        if dvemode == "ttr":
                            # tvw = tv*w, then fused (tu*tvw -> sum_p) per tile
                            tvw = actp.tile([128, fuse, G_, P], F16, tag="tvw")
                            nc.vector.tensor_tensor(tvw, tvb, wb_sb, ALU.mult)
                            scr = actp.tile([128, fuse, G_, P], F16, tag="scr")
                            for f in range(fuse):
                                for j in range(G_):
                                    nc.vector.tensor_tensor_reduce(
                                        scr[:, f, j], tub[:, f, j], tvw[:, f, j],
                                        1.0, 0.0, ALU.mult, ALU.add,
                                        alpha_sb[:, ti + f * G_ + j:
                                                 ti + f * G_ + j + 1],
                                    )
                        elif dvemode == "fold":
                            prod = actp.tile([128, fuse, G_, P], F16, tag="prod")
                            nc.vector.tensor_tensor(prod, tub, tvb, ALU.mult)
                            prodw = actp.tile([128, fuse, G_, P], F16, tag="prodw")
                            nc.vector.tensor_tensor(prodw, prod, wb_sb, ALU.mult)
                            h1 = actp.tile([128, fuse, G_, 50], F16, tag="h1")
                            nc.vector.tensor_tensor(
                                h1, prodw[:, :, :, 0:50], prodw[:, :, :, 50:100],
                                ALU.add)
                            h2 = actp.tile([128, fuse, G_, 25], F16, tag="h2")
                            nc.vector.tensor_tensor(
                                h2, h1[:, :, :, 0:25], h1[:, :, :, 25:50],
                                ALU.add)
                            nc.vector.tensor_reduce(
                                alpha_sb[:, ti:ti + fuse * G_], h2,
                                axis=mybir.AxisListType.X, op=ALU.add,
                            )
                        else:
                            prod = actp.tile([128, fuse, G_, P], F16, tag="prod")
                            nc.vector.tensor_tensor(prod, tub, tvb, ALU.mult)
                            prodw = actp.tile([128, fuse, G_, P], F16, tag="prodw")
                            nc.vector.tensor_tensor(prodw, prod, wb_sb, ALU.mult)
                            nc.vector.tensor_reduce(
                                alpha_sb[:, ti:ti + fuse * G_], prodw,
                                axis=mybir.AxisListType.X, op=ALU.add,
                            )

        # ---------------- phase 2: softmax ----------------
        if "p2" not in parts:
            with tc.tile_pool(name="p2x", bufs=1) as p2x:
                dummy = p2x.tile([128, TPC], F32)
                nc.vector.tensor_copy(dummy, alpha_sb)
                nc.sync.dma_start(out_t, dummy)
            return
        with (
            tc.tile_pool(name="p2", bufs=1) as p2,
            tc.tile_pool(name="p2ps", bufs=1, space="PSUM") as p2ps,
            tc.tile_pool(name="dram", bufs=1, space="DRAM") as dramp,
        ):
            def phase2_v2():
                # softmax without max-subtraction: |alpha| <~ sum|w| ~ 80, so
                # exp(alpha) and the per-core sums stay well inside fp32 range
                # (validated against the reference inputs in test.py).
                e_sb = p2.tile([128, TPC], F32, tag="e_sb")
                sums = p2.tile([128, 1], F32, tag="sums")
                nc.scalar.activation(e_sb, alpha_sb, AF.Exp, accum_out=sums)
                onesP = p2.tile([128, 1], F32, tag="onesP")
                nc.vector.memset(onesP, 1.0)
                s_ps = p2ps.tile([1, 1], F32, tag="s_ps")
                nc.tensor.matmul(s_ps, lhsT=sums, rhs=onesP,
                                 start=True, stop=True)
                s_sb = p2.tile([1, 1], F32, tag="s_sb")
                nc.vector.tensor_copy(s_sb, s_ps)
                s_tot = p2.tile([1, 1], F32, tag="s_tot")
                if use_collective:
                    cin = dramp.tile([1, 1], F32, tag="cin")
                    cout = dramp.tile([1, 1], F32, tag="cout")
                    nc.sync.dma_start(cin, s_sb)
                    nc.gpsimd.collective_compute(
                        "AllReduce",
                        ALU.add,
                        ins=[cin.opt()],
                        outs=[cout.opt()],
                        replica_groups=[list(range(n_cores))],
                    )
                    nc.sync.dma_start(s_tot, cout)
                else:
                    nc.vector.tensor_scalar_mul(s_tot, s_sb, float(n_cores))
                r_s = p2.tile([1, 1], F32, tag="r_s")
                nc.vector.reciprocal(r_s, s_tot)
                ones1 = p2.tile([1, 128], F32, tag="ones1")
                nc.vector.memset(ones1, 1.0)
                c_ps = p2ps.tile([128, 1], F32, tag="c_ps")
                nc.tensor.matmul(c_ps, lhsT=ones1, rhs=r_s,
                                 start=True, stop=True)
                c128 = p2.tile([128, 1], F32, tag="c128")
                nc.vector.tensor_copy(c128, c_ps)
                outt = p2.tile([128, TPC], F32, tag="outt")
                nc.vector.tensor_scalar_mul(outt, e_sb, c128)
                nc.sync.dma_start(out_t, outt)

            def phase2():
                mx = p2.tile([128, 1], F32, tag="mx")
                nc.vector.reduce_max(mx, alpha_sb, axis=mybir.AxisListType.X)
                mxr = p2.tile([128, 1], F32, tag="mxr")
                nc.gpsimd.partition_all_reduce(
                    mxr, mx, channels=128, reduce_op=bass_isa.ReduceOp.max
                )
                negm = p2.tile([128, 1], F32, tag="negm")
                nc.vector.tensor_scalar_mul(negm, mxr, -1.0)
                e_sb = p2.tile([128, TPC], F32, tag="e_sb")
                sums = p2.tile([128, 1], F32, tag="sums")
                nc.scalar.activation(e_sb, alpha_sb, AF.Exp, bias=negm,
                                     accum_out=sums)
                sr = p2.tile([128, 1], F32, tag="sr")
                nc.gpsimd.partition_all_reduce(
                    sr, sums, channels=128, reduce_op=bass_isa.ReduceOp.add
                )
                # pack (m_local, s_local) and all-gather across cores
                ms = p2.tile([1, 2], F32, tag="ms")
                nc.vector.tensor_copy(ms[:, 0:1], mxr[0:1, :])
                nc.vector.tensor_copy(ms[:, 1:2], sr[0:1, :])
                gath = p2.tile([1, 2, n_cores], F32, tag="gath")
                if use_collective:
                    cin = dramp.tile([1, 2], F32, tag="cin")
                    cout = dramp.tile([n_cores, 2], F32, tag="cout")
                    nc.sync.dma_start(cin, ms)
                    nc.gpsimd.collective_compute(
                        "AllGather",
                        ALU.bypass,
                        ins=[cin.opt()],
                        outs=[cout.opt()],
                        replica_groups=[list(range(n_cores))],
                    )
                    nc.sync.dma_start(gath, cout.rearrange("i two -> two i"))
                else:
                    # single-core debug: replicate local (m, s) n_cores times
                    for i in range(n_cores):
                        nc.vector.tensor_copy(gath[:, :, i], ms)
                mg = p2.tile([1, 1], F32, tag="mg")
                nc.vector.reduce_max(mg, gath[:, 0], axis=mybir.AxisListType.X)
                neg_mg = p2.tile([1, 1], F32, tag="neg_mg")
                nc.vector.tensor_scalar_mul(neg_mg, mg, -1.0)
                e8 = p2.tile([1, n_cores], F32, tag="e8")
                nc.scalar.activation(e8, gath[:, 0], AF.Exp, bias=neg_mg)
                p8 = p2.tile([1, n_cores], F32, tag="p8")
                nc.vector.tensor_tensor(p8, e8, gath[:, 1], ALU.mult)
                s_tot = p2.tile([1, 1], F32, tag="s_tot")
                nc.vector.reduce_sum(s_tot, p8, axis=mybir.AxisListType.X)
                r_s = p2.tile([1, 1], F32, tag="r_s")
                nc.vector.reciprocal(r_s, s_tot)
                eml = p2.tile([1, 1], F32, tag="eml")
                nc.scalar.activation(eml, mxr[0:1, :], AF.Exp, bias=neg_mg)
                c1 = p2.tile([1, 1], F32, tag="c1")
                nc.vector.tensor_tensor(c1, eml, r_s, ALU.mult)
                c128 = p2.tile([128, 1], F32, tag="c128")
                nc.gpsimd.partition_broadcast(c128, c1)
                outt = p2.tile([128, TPC], F32, tag="outt")
                nc.vector.tensor_scalar_mul(outt, e_sb, c128)
                nc.sync.dma_start(out_t, outt)

            for _ in range(repeats if "p2rep" in parts else 1):
                phase2_v2() if p2v == 2 else phase2()


_CACHE: dict = {}


def _get_nc() -> bass.Bass:
    if "nc" not in _CACHE:
        _CACHE["nc"] = _build(NCORES)
    return _CACHE["nc"]


def kernel(features: np.ndarray, u: np.ndarray, v: np.ndarray, w: np.ndarray,
           **_unused) -> np.ndarray:
    features = np.asarray(features)
    u, v, w = np.asarray(u), np.asarray(v), np.asarray(w)
    assert features.shape == (B, C)
    f16 = features.astype(np.float16)
    uvt = np.ascontiguousarray(
        np.concatenate([u.T, v.T], axis=1).astype(np.float16)
    )  # [C, 2P]
    uv_arr = np.ascontiguousarray(uvt.reshape(2, 128, 2 * P))
    wb = np.ascontiguousarray(
        np.tile(w.reshape(-1).astype(np.float16), (128, FUSE * G))
    )  # [128, FUSE*G*P]
    in_maps = []
    for i in range(NCORES):
        sl = slice(i * BS, (i + 1) * BS)
        in_maps.append({
            "f0": np.ascontiguousarray(f16[sl, :128].T),
            "f1": np.ascontiguousarray(f16[sl, 128:].T),
            "uv": uv_arr,
            "wb": wb,
        })
    res = run_bass_kernel_spmd(_get_nc(), in_maps, core_ids=list(range(NCORES)))
    outs = [r["out"] for r in res.results]
    return np.concatenate([o.T.reshape(-1) for o in outs]).astype(np.float32)

